# revision 8
# baseline (speedup 1.0000x reference)
"""Causal GQA attention (B=2, T=2048, D=2048, QH=16, KVH=4, HD=128) on 8 TRN2 cores.

Sharding: DP-2 over batch x TP-4 over KV-head groups.
  core c -> batch c//4, kv head c%4, q heads 4*(c%4)..4*(c%4)+3.
Each core computes a partial (T, D) output (its heads' contribution through wo);
the host sums the 4 partials per batch (the all-reduce of the "wo along in dim"
sharding) and stacks the two batches.

Device dataflow (everything transposed; no on-device activation transposes):
  - host feeds xT = x[b].T                            (D, T)
  - qT/kT = W^T x computed directly in [hd, t] layout (wq chunks are lhsT)
  - RoPE via swap-permutation matmul (rot = R @ qT) + DVE mul/add with
    host cos / sign-folded-sin tables in [hd, t] layout
  - S^T[key, q] = (kT_blk)^T @ qT  per 128-key block  (one matmul, K=hd=128)
  - exp on ACT with fused 1/sqrt(hd) scale, PSUM -> SBUF f32r
  - causal: fully-masked column ranges of diagonal blocks are never computed;
    the 128x128 diagonal triangle is masked by a DVE multiply
  - O^T[hd, q] += V_blk^T @ expS^T   (V natural from 4 PE transposes per tile)
  - den[1, q]  += ones^T @ expS^T    (M=1 matmul, same rhs)
  - normalize (deferred one head so PE never stalls): den -> ACT evac ->
    PE broadcast to [128,512] -> ACT evac -> DVE divide O^T / den_bcast
  - out[t, d] = sum_h (OT_h)^T @ wo_h  accumulated over the 4 heads

All matmuls run in float32r (full PE rate at N=512, ~1e-4 rel err).
"""
import numpy as np
from contextlib import ExitStack

import concourse.bacc as bacc
import concourse.tile as tile
import concourse.mybir as mybir
from concourse.bass_utils import run_bass_kernel_spmd

B, T, D = 2, 2048, 2048
QH, KVH = 16, 4
HD = D // QH            # 128
P = 128
NT = T // 512           # 4 t-tiles of 512
DC = D // P             # 16 contraction chunks
KB = T // P             # 16 key blocks
F32 = mybir.dt.float32
F32R = mybir.dt.float32r
AF = mybir.ActivationFunctionType
ALU = mybir.AluOpType
SCALE = float(1.0 / np.sqrt(HD))

_cached = {}


def _build():
    nc = bacc.Bacc("TRN2", target_bir_lowering=False, debug=False)
    xT = nc.dram_tensor("xT", [D, T], F32R, kind="ExternalInput")
    wq = nc.dram_tensor("wq", [D, 4 * HD], F32R, kind="ExternalInput")
    wk = nc.dram_tensor("wk", [D, HD], F32R, kind="ExternalInput")
    wv = nc.dram_tensor("wv", [D, HD], F32R, kind="ExternalInput")
    wo = nc.dram_tensor("wo", [4 * HD, D], F32R, kind="ExternalInput")
    cosT = nc.dram_tensor("cosT", [HD, T], F32R, kind="ExternalInput")
    ssinT = nc.dram_tensor("ssinT", [HD, T], F32R, kind="ExternalInput")
    rmat = nc.dram_tensor("rmat", [P, P], F32R, kind="ExternalInput")
    tri = nc.dram_tensor("tri", [P, P], F32R, kind="ExternalInput")
    ident = nc.dram_tensor("ident", [P, P], F32R, kind="ExternalInput")
    out = nc.dram_tensor("out", [T, D], F32, kind="ExternalOutput")

    with tile.TileContext(nc) as tc, ExitStack() as ctx:
        const = ctx.enter_context(tc.tile_pool(name="const", bufs=1))
        kvres = ctx.enter_context(tc.tile_pool(name="kvres", bufs=1))
        xc_pool = ctx.enter_context(tc.tile_pool(name="xc", bufs=16))
        qr_pool = ctx.enter_context(tc.tile_pool(name="qr", bufs=5))
        tmp_pool = ctx.enter_context(tc.tile_pool(name="tmp", bufs=2))
        e_pool = ctx.enter_context(tc.tile_pool(name="ep", bufs=3))
        ot_pool = ctx.enter_context(tc.tile_pool(name="ot", bufs=1))
        oev_pool = ctx.enter_context(tc.tile_pool(name="oev", bufs=2))
        bc_pool = ctx.enter_context(tc.tile_pool(name="bc", bufs=2))
        sm_pool = ctx.enter_context(tc.tile_pool(name="sm", bufs=2))

        ps_w = ctx.enter_context(tc.tile_pool(name="psw", bufs=4, space="PSUM"))
        ps_o = ctx.enter_context(tc.tile_pool(name="pso", bufs=2, space="PSUM"))
        ps_d = ctx.enter_context(tc.tile_pool(name="psd", bufs=2, space="PSUM"))

        # ---- resident constants (split + ordered for startup overlap) ----
        wq_sb = const.tile([P, DC, 4 * HD], F32R, tag="wq")
        wk_sb = const.tile([P, DC, HD], F32R, tag="wk")
        wv_sb = const.tile([P, DC, HD], F32R, tag="wv")
        wo_sb = const.tile([P, 4, D], F32R, tag="wo")
        cos_sb = const.tile([P, T], F32R, tag="cos")
        sin_sb = const.tile([P, T], F32R, tag="sin")
        rm_sb = const.tile([P, P], F32R, tag="rm")
        tri_sb = const.tile([P, P], F32R, tag="tri")
        id_sb = const.tile([P, P], F32R, tag="id")

        kT_all = kvres.tile([P, T], F32R, tag="kT")
        v_all = kvres.tile([P, KB, HD], F32R, tag="V")

        xT_v = xT.rearrange("(dc p) t -> dc p t", p=P)
        wq_v = wq.rearrange("(dc p) n -> dc p n", p=P)
        wk_v = wk.rearrange("(dc p) n -> dc p n", p=P)
        wv_v = wv.rearrange("(dc p) n -> dc p n", p=P)
        wo_v = wo.rearrange("(c p) n -> c p n", p=P)

        # first t-tile's x chunks interleaved with wq chunks, then the rest
        xcs0 = []
        for dc in range(DC):
            xc = xc_pool.tile([P, 512], F32R, tag="xc", name=f"xc0_{dc}")
            nc.sync.dma_start(out=xc[:], in_=xT_v[dc, :, 0:512])
            xcs0.append(xc)
            nc.sync.dma_start(out=wq_sb[:, dc, :], in_=wq_v[dc])
        nc.sync.dma_start(out=cos_sb[:], in_=cosT[:])
        nc.sync.dma_start(out=sin_sb[:], in_=ssinT[:])
        for dc in range(DC):
            nc.sync.dma_start(out=wk_sb[:, dc, :], in_=wk_v[dc])
            nc.sync.dma_start(out=wv_sb[:, dc, :], in_=wv_v[dc])
        nc.sync.dma_start(out=rm_sb[:], in_=rmat[:])
        nc.sync.dma_start(out=tri_sb[:], in_=tri[:])
        nc.sync.dma_start(out=id_sb[:], in_=ident[:])
        for c in range(4):
            nc.sync.dma_start(out=wo_sb[:, c, :], in_=wo_v[c])

        ones_col = tri_sb[:, P - 1:P]   # [128,1] of ones
        ones_row = tri_sb[0:1, :]       # [1,128] of ones

        def rope(dst_ap, src_ps, tt, nm):
            """dst[hd, 512] = src*cos + (R@src)*ssin for t-tile tt. src is PSUM."""
            c_sl = cos_sb[:, tt * 512:(tt + 1) * 512]
            s_sl = sin_sb[:, tt * 512:(tt + 1) * 512]
            sb = tmp_pool.tile([P, 512], F32R, tag="evac", name=f"ev_{nm}")
            nc.scalar.copy(sb[:], src_ps[:])
            rot_ps = ps_w.tile([P, 512], F32, tag="w", name=f"rot_{nm}")
            nc.tensor.matmul(rot_ps[:], rm_sb[:], sb[:], start=True, stop=True)
            t1 = tmp_pool.tile([P, 512], F32, tag="t1", name=f"t1_{nm}")
            nc.vector.tensor_mul(t1[:], sb[:], c_sl)
            t2 = tmp_pool.tile([P, 512], F32, tag="t2", name=f"t2_{nm}")
            nc.vector.tensor_mul(t2[:], rot_ps[:], s_sl)
            with nc.allow_low_precision(reason="f32r rounding for PE"):
                nc.vector.tensor_add(dst_ap, t1[:], t2[:])

        for tt in range(NT):
            tsl = slice(tt * 512, (tt + 1) * 512)
            # ---------- Phase A: projections (groups of 2 PSUM tiles) ----------
            if tt == 0:
                xcs = xcs0
            else:
                xcs = []
                for dc in range(DC):
                    xc = xc_pool.tile([P, 512], F32R, tag="xc", name=f"xc{tt}_{dc}")
                    nc.sync.dma_start(out=xc[:], in_=xT_v[dc, :, tsl])
                    xcs.append(xc)

            qT_roped = [qr_pool.tile([P, 512], F32R, tag="qr", name=f"qr{tt}_{i}")
                        for i in range(4)]
            groups = [[("q", 0), ("q", 1)], [("q", 2), ("q", 3)], [("k", 0), ("v", 0)]]
            pending_rope = []
            for gi, grp in enumerate(groups):
                pss = {}
                for tgt in grp:
                    pss[tgt] = ps_w.tile([P, 512], F32, tag="w",
                                         name=f"proj{tt}_{tgt[0]}{tgt[1]}")
                for dc in range(DC):
                    for tgt in grp:
                        kind, idx = tgt
                        if kind == "q":
                            lhsT = wq_sb[:, dc, idx * HD:(idx + 1) * HD]
                        elif kind == "k":
                            lhsT = wk_sb[:, dc, :]
                        else:
                            lhsT = wv_sb[:, dc, :]
                        nc.tensor.matmul(pss[tgt][:], lhsT, xcs[dc][:],
                                         start=(dc == 0), stop=(dc == DC - 1))
                # emit previous group's rope now (its ACT evac overlapped this group)
                for kind, idx, ps in pending_rope:
                    if kind == "q":
                        rope(qT_roped[idx][:], ps, tt, f"q{tt}_{idx}")
                    else:
                        rope(kT_all[:, tsl], ps, tt, f"k{tt}")
                pending_rope = []
                for tgt in grp:
                    kind, idx = tgt
                    if kind in ("q", "k"):
                        pending_rope.append((kind, idx, pss[tgt]))
                    else:
                        vt_sb = tmp_pool.tile([P, 512], F32R, tag="evac",
                                              name=f"vt{tt}")
                        nc.scalar.copy(vt_sb[:], pss[tgt][:])
                        tr_ps = ps_w.tile([P, 512], F32R, tag="w", name=f"vtr{tt}")
                        for i in range(4):
                            nc.tensor.transpose(tr_ps[:, i * P:(i + 1) * P],
                                                vt_sb[:, i * P:(i + 1) * P], id_sb[:])
                        for i in range(4):
                            with nc.allow_low_precision(reason="f32r store"):
                                nc.vector.tensor_copy(v_all[:, tt * 4 + i, :],
                                                      tr_ps[:, i * P:(i + 1) * P])
            for kind, idx, ps in pending_rope:
                if kind == "q":
                    rope(qT_roped[idx][:], ps, tt, f"q{tt}_{idx}")
                else:
                    rope(kT_all[:, tsl], ps, tt, f"k{tt}")

            # ---------- Phase B: attention, one-head-deferred normalization ----
            nkb = 4 * (tt + 1)
            ot_sb = ot_pool.tile([P, 4, 512], F32R, tag="ot", name=f"ot{tt}")
            pending_norm = []

            def finish_head(o_ps, den_sb, hh, tt=tt, ot_sb=ot_sb):
                ln_sb = sm_pool.tile([1, 512], F32, tag="ln", name=f"ln{tt}_{hh}")
                nc.scalar.activation(ln_sb[:], den_sb[:], AF.Ln)
                recip_sb = sm_pool.tile([1, 512], F32R, tag="recip",
                                        name=f"rc{tt}_{hh}")
                # 1/x = exp(-ln x); Ln and Exp share one ACT table set
                nc.scalar.activation(recip_sb[:], ln_sb[:], AF.Exp, scale=-1.0)
                bc_ps = ps_w.tile([P, 512], F32, tag="w", name=f"bc{tt}_{hh}")
                nc.tensor.matmul(bc_ps[:], ones_row, recip_sb[:], start=True, stop=True)
                bc_sb = bc_pool.tile([P, 512], F32R, tag="bc", name=f"bs{tt}_{hh}")
                nc.scalar.copy(bc_sb[:], bc_ps[:])
                with nc.allow_low_precision(reason="norm"):
                    nc.vector.tensor_mul(ot_sb[:, hh, :], o_ps[:], bc_sb[:])

            for hh in range(4):
                o_ps = ps_o.tile([P, 512], F32, tag="o", name=f"o{tt}_{hh}")
                den_ps = ps_d.tile([1, 512], F32, tag="den", name=f"d{tt}_{hh}")
                prev = None   # (kb, lo, e_sb)
                for kb in range(nkb):
                    di = kb - 4 * tt          # >=0 on diagonal blocks
                    lo = di * P if di > 0 else 0
                    s_ps = ps_w.tile([P, 512], F32, tag="w", name=f"s{tt}_{hh}_{kb}")
                    nc.tensor.matmul(s_ps[:, lo:512],
                                     kT_all[:, kb * P:(kb + 1) * P],
                                     qT_roped[hh][:, lo:512], start=True, stop=True)
                    e_sb = e_pool.tile([P, 512], F32R, tag="e", name=f"e{tt}_{hh}_{kb}")
                    nc.scalar.activation(e_sb[:, lo:512], s_ps[:, lo:512], AF.Exp,
                                         scale=SCALE)
                    if di >= 0:
                        with nc.allow_low_precision(reason="mask mult"):
                            nc.vector.tensor_mul(e_sb[:, di * P:(di + 1) * P],
                                                 e_sb[:, di * P:(di + 1) * P],
                                                 tri_sb[:])
                    if prev is not None:
                        pkb, plo, pe = prev
                        nc.tensor.matmul(o_ps[:, plo:512], v_all[:, pkb, :],
                                         pe[:, plo:512],
                                         start=(pkb == 0), stop=False)
                        nc.tensor.matmul(den_ps[:, plo:512], ones_col,
                                         pe[:, plo:512],
                                         start=(pkb == 0), stop=False)
                    prev = (kb, lo, e_sb)
                pkb, plo, pe = prev
                nc.tensor.matmul(o_ps[:, plo:512], v_all[:, pkb, :], pe[:, plo:512],
                                 start=(pkb == 0), stop=True)
                nc.tensor.matmul(den_ps[:, plo:512], ones_col, pe[:, plo:512],
                                 start=(pkb == 0), stop=True)
                # evac den now (ACT, off the PE path); defer bc+divide one head
                den_sb = sm_pool.tile([1, 512], F32R, tag="den", name=f"dn{tt}_{hh}")
                nc.scalar.copy(den_sb[:], den_ps[:])
                if pending_norm:
                    finish_head(*pending_norm.pop())
                pending_norm.append((o_ps, den_sb, hh))
            finish_head(*pending_norm.pop())

            # ---------- Phase C: output projection ----------
            for tc4 in range(4):
                trow = tt * 512 + tc4 * P
                for doc in range(4):
                    f_ps = ps_w.tile([P, 512], F32, tag="w", name=f"f{tt}_{tc4}_{doc}")
                    for hh in range(4):
                        nc.tensor.matmul(f_ps[:],
                                         ot_sb[:, hh, tc4 * P:(tc4 + 1) * P],
                                         wo_sb[:, hh, doc * 512:(doc + 1) * 512],
                                         start=(hh == 0), stop=(hh == 3))
                    o_ev = oev_pool.tile([P, 512], F32, tag="oev",
                                         name=f"oe{tt}_{tc4}_{doc}")
                    nc.vector.tensor_copy(o_ev[:], f_ps[:])
                    nc.sync.dma_start(out=out[trow:trow + P, doc * 512:(doc + 1) * 512],
                                      in_=o_ev[:])
    nc.compile()
    return nc


def _host_tables():
    freqs = (1.0 / (np.float32(10000.0) **
                    (np.arange(0, HD, 2, dtype=np.float32) / np.float32(HD)))).astype(np.float32)
    t = np.arange(T, dtype=np.float32)
    ang = t[:, None] * freqs[None, :]
    cos = np.tile(np.cos(ang), (1, 2)).astype(np.float32)   # (T, HD)
    sin = np.tile(np.sin(ang), (1, 2)).astype(np.float32)
    cosT = np.ascontiguousarray(cos.T)                       # (HD, T)
    sinT = np.ascontiguousarray(sin.T)
    ssinT = sinT.copy()
    ssinT[:HD // 2] *= -1.0                                  # sign-folded sin
    # pure half-swap permutation; the rotate-half sign lives in ssinT
    rmat = np.zeros((P, P), dtype=np.float32)
    for j in range(HD // 2):
        rmat[j + HD // 2, j] = 1.0
    for j in range(HD // 2, HD):
        rmat[j - HD // 2, j] = 1.0
    tri = (np.arange(P)[:, None] <= np.arange(P)[None, :]).astype(np.float32)
    ident = np.eye(P, dtype=np.float32)
    return cosT, ssinT, rmat, tri, ident


def kernel(x, wq, wk, wv, wo):
    if "nc" not in _cached:
        _cached["nc"] = _build()
    nc = _cached["nc"]
    cosT, ssinT, rmat, tri, ident = _host_tables()
    x = np.asarray(x, dtype=np.float32)
    wq = np.asarray(wq, dtype=np.float32)
    wk = np.asarray(wk, dtype=np.float32)
    wv = np.asarray(wv, dtype=np.float32)
    wo = np.asarray(wo, dtype=np.float32)

    in_maps = []
    for c in range(8):
        b, h = divmod(c, 4)
        in_maps.append({
            "xT": np.ascontiguousarray(x[b].T),
            "wq": np.ascontiguousarray(wq[:, h * 512:(h + 1) * 512]),
            "wk": np.ascontiguousarray(wk[:, h * HD:(h + 1) * HD]),
            "wv": np.ascontiguousarray(wv[:, h * HD:(h + 1) * HD]),
            "wo": np.ascontiguousarray(wo[h * 512:(h + 1) * 512, :]),
            "cosT": cosT, "ssinT": ssinT, "rmat": rmat, "tri": tri, "ident": ident,
        })
    res = run_bass_kernel_spmd(nc, in_maps, core_ids=list(range(8)))
    outs = [res.results[c]["out"] for c in range(8)]
    full = np.stack([outs[0] + outs[1] + outs[2] + outs[3],
                     outs[4] + outs[5] + outs[6] + outs[7]], axis=0)
    return full.astype(np.float32)


# revision 9
# speedup vs baseline: 1.1133x; 1.1133x over previous
"""Causal GQA attention (B=2, T=2048, D=2048, QH=16, KVH=4, HD=128) on 8 TRN2 cores.

Sharding: DP-2 over batch x TP-4 over KV-head groups.
  core c -> batch c//4, kv head c%4, q heads 4*(c%4)..4*(c%4)+3.
Each core computes a partial (T, D) output (its heads' contribution through wo);
the host sums the 4 partials per batch (the all-reduce of the "wo along in dim"
sharding) and stacks the two batches.

Device dataflow (everything transposed; no on-device activation transposes):
  - host feeds xT = x[b].T                            (D, T)
  - qT/kT = W^T x computed directly in [hd, t] layout (wq chunks are lhsT)
  - RoPE via swap-permutation matmul (rot = R @ qT) + DVE mul/add with
    host cos / sign-folded-sin tables in [hd, t] layout
  - S^T[key, q] = (kT_blk)^T @ qT  per 128-key block  (one matmul, K=hd=128)
  - exp on ACT with fused 1/sqrt(hd) scale, PSUM -> SBUF f32r
  - causal: fully-masked column ranges of diagonal blocks are never computed;
    the 128x128 diagonal triangle is masked by a DVE multiply
  - O^T[hd, q] += V_blk^T @ expS^T   (V natural from 4 PE transposes per tile)
  - den[1, q]  += ones^T @ expS^T    (M=1 matmul, same rhs)
  - normalize (deferred one head so PE never stalls): den -> ACT evac ->
    PE broadcast to [128,512] -> ACT evac -> DVE divide O^T / den_bcast
  - out[t, d] = sum_h (OT_h)^T @ wo_h  accumulated over the 4 heads

All matmuls run in float32r (full PE rate at N=512, ~1e-4 rel err).
"""
import numpy as np
from contextlib import ExitStack

import concourse.bacc as bacc
import concourse.tile as tile
import concourse.mybir as mybir
from concourse.bass_utils import run_bass_kernel_spmd

B, T, D = 2, 2048, 2048
QH, KVH = 16, 4
HD = D // QH            # 128
P = 128
NT = T // 512           # 4 t-tiles of 512
DC = D // P             # 16 contraction chunks
KB = T // P             # 16 key blocks
F32 = mybir.dt.float32
F32R = mybir.dt.float32r
AF = mybir.ActivationFunctionType
ALU = mybir.AluOpType
SCALE = float(1.0 / np.sqrt(HD))

_cached = {}


def _build():
    nc = bacc.Bacc("TRN2", target_bir_lowering=False, debug=False)
    xT = nc.dram_tensor("xT", [D, T], F32R, kind="ExternalInput")
    wq = nc.dram_tensor("wq", [D, 4 * HD], F32R, kind="ExternalInput")
    wk = nc.dram_tensor("wk", [D, HD], F32R, kind="ExternalInput")
    wv = nc.dram_tensor("wv", [D, HD], F32R, kind="ExternalInput")
    wo = nc.dram_tensor("wo", [4 * HD, D], F32R, kind="ExternalInput")
    cosT = nc.dram_tensor("cosT", [HD, T], F32R, kind="ExternalInput")
    ssinT = nc.dram_tensor("ssinT", [HD, T], F32R, kind="ExternalInput")
    rmat = nc.dram_tensor("rmat", [P, P], F32R, kind="ExternalInput")
    tri = nc.dram_tensor("tri", [P, P], F32R, kind="ExternalInput")
    ident = nc.dram_tensor("ident", [P, P], F32R, kind="ExternalInput")
    out = nc.dram_tensor("out", [T, D], F32, kind="ExternalOutput")

    with tile.TileContext(nc) as tc, ExitStack() as ctx:
        const = ctx.enter_context(tc.tile_pool(name="const", bufs=1))
        kvres = ctx.enter_context(tc.tile_pool(name="kvres", bufs=1))
        xc_pool = ctx.enter_context(tc.tile_pool(name="xc", bufs=16))
        qr_pool = ctx.enter_context(tc.tile_pool(name="qr", bufs=5))
        tmp_pool = ctx.enter_context(tc.tile_pool(name="tmp", bufs=2))
        e_pool = ctx.enter_context(tc.tile_pool(name="ep", bufs=3))
        ot_pool = ctx.enter_context(tc.tile_pool(name="ot", bufs=1))
        oev_pool = ctx.enter_context(tc.tile_pool(name="oev", bufs=2))
        bc_pool = ctx.enter_context(tc.tile_pool(name="bc", bufs=2))
        sm_pool = ctx.enter_context(tc.tile_pool(name="sm", bufs=2))

        ps_w = ctx.enter_context(tc.tile_pool(name="psw", bufs=4, space="PSUM"))
        ps_o = ctx.enter_context(tc.tile_pool(name="pso", bufs=2, space="PSUM"))
        ps_d = ctx.enter_context(tc.tile_pool(name="psd", bufs=2, space="PSUM"))

        # ---- resident constants (split + ordered for startup overlap) ----
        wq_sb = const.tile([P, DC, 4 * HD], F32R, tag="wq")
        wk_sb = const.tile([P, DC, HD], F32R, tag="wk")
        wv_sb = const.tile([P, DC, HD], F32R, tag="wv")
        wo_sb = const.tile([P, 4, D], F32R, tag="wo")
        cos_sb = const.tile([P, T], F32R, tag="cos")
        sin_sb = const.tile([P, T], F32R, tag="sin")
        rm_sb = const.tile([P, P], F32R, tag="rm")
        tri_sb = const.tile([P, P], F32R, tag="tri")
        id_sb = const.tile([P, P], F32R, tag="id")

        kT_all = kvres.tile([P, T], F32R, tag="kT")
        v_all = kvres.tile([P, KB, HD], F32R, tag="V")

        xT_v = xT.rearrange("(dc p) t -> dc p t", p=P)
        wq_v = wq.rearrange("(dc p) n -> dc p n", p=P)
        wk_v = wk.rearrange("(dc p) n -> dc p n", p=P)
        wv_v = wv.rearrange("(dc p) n -> dc p n", p=P)
        wo_v = wo.rearrange("(c p) n -> c p n", p=P)

        # first t-tile's x chunks interleaved with wq chunks, then the rest
        xcs0 = []
        for dc in range(DC):
            xc = xc_pool.tile([P, 512], F32R, tag="xc", name=f"xc0_{dc}")
            nc.sync.dma_start(out=xc[:], in_=xT_v[dc, :, 0:512])
            xcs0.append(xc)
            nc.sync.dma_start(out=wq_sb[:, dc, :], in_=wq_v[dc])
        nc.sync.dma_start(out=cos_sb[:], in_=cosT[:])
        nc.sync.dma_start(out=sin_sb[:], in_=ssinT[:])
        for dc in range(DC):
            nc.sync.dma_start(out=wk_sb[:, dc, :], in_=wk_v[dc])
            nc.sync.dma_start(out=wv_sb[:, dc, :], in_=wv_v[dc])
        nc.sync.dma_start(out=rm_sb[:], in_=rmat[:])
        nc.sync.dma_start(out=tri_sb[:], in_=tri[:])
        nc.sync.dma_start(out=id_sb[:], in_=ident[:])
        ones_col = tri_sb[:, P - 1:P]   # [128,1] of ones
        ones_row = tri_sb[0:1, :]       # [1,128] of ones

        def rope(dst_ap, src_ps, tt, nm):
            """dst[hd, 512] = src*cos + (R@src)*ssin for t-tile tt. src is PSUM."""
            c_sl = cos_sb[:, tt * 512:(tt + 1) * 512]
            s_sl = sin_sb[:, tt * 512:(tt + 1) * 512]
            sb = tmp_pool.tile([P, 512], F32R, tag="evac", name=f"ev_{nm}")
            nc.scalar.copy(sb[:], src_ps[:])
            rot_ps = ps_w.tile([P, 512], F32, tag="w", name=f"rot_{nm}")
            nc.tensor.matmul(rot_ps[:], rm_sb[:], sb[:], start=True, stop=True)
            t1 = tmp_pool.tile([P, 512], F32, tag="t1", name=f"t1_{nm}")
            nc.vector.tensor_mul(t1[:], sb[:], c_sl)
            t2 = tmp_pool.tile([P, 512], F32, tag="t2", name=f"t2_{nm}")
            nc.vector.tensor_mul(t2[:], rot_ps[:], s_sl)
            with nc.allow_low_precision(reason="f32r rounding for PE"):
                nc.vector.tensor_add(dst_ap, t1[:], t2[:])

        for tt in range(NT):
            tsl = slice(tt * 512, (tt + 1) * 512)
            # ---------- Phase A: projections (groups of 2 PSUM tiles) ----------
            if tt == 0:
                xcs = xcs0
            else:
                xcs = []
                for dc in range(DC):
                    xc = xc_pool.tile([P, 512], F32R, tag="xc", name=f"xc{tt}_{dc}")
                    nc.sync.dma_start(out=xc[:], in_=xT_v[dc, :, tsl])
                    xcs.append(xc)

            qT_roped = [qr_pool.tile([P, 512], F32R, tag="qr", name=f"qr{tt}_{i}")
                        for i in range(4)]
            groups = [[("q", 0), ("q", 1)], [("q", 2), ("q", 3)], [("k", 0), ("v", 0)]]
            pending_rope = []
            for gi, grp in enumerate(groups):
                pss = {}
                for tgt in grp:
                    pss[tgt] = ps_w.tile([P, 512], F32, tag="w",
                                         name=f"proj{tt}_{tgt[0]}{tgt[1]}")
                for dc in range(DC):
                    for tgt in grp:
                        kind, idx = tgt
                        if kind == "q":
                            lhsT = wq_sb[:, dc, idx * HD:(idx + 1) * HD]
                        elif kind == "k":
                            lhsT = wk_sb[:, dc, :]
                        else:
                            lhsT = wv_sb[:, dc, :]
                        nc.tensor.matmul(pss[tgt][:], lhsT, xcs[dc][:],
                                         start=(dc == 0), stop=(dc == DC - 1))
                # emit previous group's rope now (its ACT evac overlapped this group)
                for kind, idx, ps in pending_rope:
                    if kind == "q":
                        rope(qT_roped[idx][:], ps, tt, f"q{tt}_{idx}")
                    else:
                        rope(kT_all[:, tsl], ps, tt, f"k{tt}")
                pending_rope = []
                for tgt in grp:
                    kind, idx = tgt
                    if kind in ("q", "k"):
                        pending_rope.append((kind, idx, pss[tgt]))
                    else:
                        vt_sb = tmp_pool.tile([P, 512], F32R, tag="evac",
                                              name=f"vt{tt}")
                        nc.scalar.copy(vt_sb[:], pss[tgt][:])
                        tr_ps = ps_w.tile([P, 512], F32R, tag="w", name=f"vtr{tt}")
                        for i in range(4):
                            nc.tensor.transpose(tr_ps[:, i * P:(i + 1) * P],
                                                vt_sb[:, i * P:(i + 1) * P], id_sb[:])
                        for i in range(4):
                            with nc.allow_low_precision(reason="f32r store"):
                                nc.vector.tensor_copy(v_all[:, tt * 4 + i, :],
                                                      tr_ps[:, i * P:(i + 1) * P])
            for kind, idx, ps in pending_rope:
                if kind == "q":
                    rope(qT_roped[idx][:], ps, tt, f"q{tt}_{idx}")
                else:
                    rope(kT_all[:, tsl], ps, tt, f"k{tt}")

            # ---------- Phase B: attention, one-head-deferred normalization ----
            nkb = 4 * (tt + 1)
            ot_sb = ot_pool.tile([P, 4, 512], F32R, tag="ot", name=f"ot{tt}")
            pending_norm = []

            def finish_head(o_ps, recip_sb, hh, tt=tt, ot_sb=ot_sb):
                bc_ps = ps_w.tile([P, 512], F32, tag="w", name=f"bc{tt}_{hh}")
                nc.tensor.matmul(bc_ps[:], ones_row, recip_sb[:], start=True, stop=True)
                bc_sb = bc_pool.tile([P, 512], F32R, tag="bc", name=f"bs{tt}_{hh}")
                nc.scalar.copy(bc_sb[:], bc_ps[:])
                with nc.allow_low_precision(reason="norm"):
                    nc.vector.tensor_mul(ot_sb[:, hh, :], o_ps[:], bc_sb[:])

            for hh in range(4):
                o_ps = ps_o.tile([P, 512], F32, tag="o", name=f"o{tt}_{hh}")
                den_ps = ps_d.tile([1, 512], F32, tag="den", name=f"d{tt}_{hh}")
                prev = None   # (kb, lo, e_sb)
                for kb in range(nkb):
                    di = kb - 4 * tt          # >=0 on diagonal blocks
                    lo = di * P if di > 0 else 0
                    s_ps = ps_w.tile([P, 512], F32, tag="w", name=f"s{tt}_{hh}_{kb}")
                    nc.tensor.matmul(s_ps[:, lo:512],
                                     kT_all[:, kb * P:(kb + 1) * P],
                                     qT_roped[hh][:, lo:512], start=True, stop=True)
                    e_sb = e_pool.tile([P, 512], F32R, tag="e", name=f"e{tt}_{hh}_{kb}")
                    nc.scalar.activation(e_sb[:, lo:512], s_ps[:, lo:512], AF.Exp,
                                         scale=SCALE)
                    if di >= 0:
                        with nc.allow_low_precision(reason="mask mult"):
                            nc.vector.tensor_mul(e_sb[:, di * P:(di + 1) * P],
                                                 e_sb[:, di * P:(di + 1) * P],
                                                 tri_sb[:])
                    if prev is not None:
                        pkb, plo, pe = prev
                        nc.tensor.matmul(o_ps[:, plo:512], v_all[:, pkb, :],
                                         pe[:, plo:512],
                                         start=(pkb == 0), stop=False)
                        nc.tensor.matmul(den_ps[:, plo:512], ones_col,
                                         pe[:, plo:512],
                                         start=(pkb == 0), stop=False)
                    prev = (kb, lo, e_sb)
                pkb, plo, pe = prev
                nc.tensor.matmul(o_ps[:, plo:512], v_all[:, pkb, :], pe[:, plo:512],
                                 start=(pkb == 0), stop=True)
                nc.tensor.matmul(den_ps[:, plo:512], ones_col, pe[:, plo:512],
                                 start=(pkb == 0), stop=True)
                # reciprocal now (DVE, off the PE path); defer bc+norm one head
                recip_sb = sm_pool.tile([1, 512], F32R, tag="recip",
                                        name=f"rc{tt}_{hh}")
                with nc.allow_low_precision(reason="recip"):
                    nc.vector.reciprocal(recip_sb[:], den_ps[:])
                if pending_norm:
                    finish_head(*pending_norm.pop())
                pending_norm.append((o_ps, recip_sb, hh))
            finish_head(*pending_norm.pop())

            if tt == 0:
                # wo arrives late on purpose: keeps startup DMA bandwidth for
                # the tensors the first projections need
                for c in range(4):
                    nc.sync.dma_start(out=wo_sb[:, c, :], in_=wo_v[c])

            # ---------- Phase C: output projection ----------
            for tc4 in range(4):
                trow = tt * 512 + tc4 * P
                for doc in range(4):
                    f_ps = ps_w.tile([P, 512], F32, tag="w", name=f"f{tt}_{tc4}_{doc}")
                    for hh in range(4):
                        nc.tensor.matmul(f_ps[:],
                                         ot_sb[:, hh, tc4 * P:(tc4 + 1) * P],
                                         wo_sb[:, hh, doc * 512:(doc + 1) * 512],
                                         start=(hh == 0), stop=(hh == 3))
                    o_ev = oev_pool.tile([P, 512], F32, tag="oev",
                                         name=f"oe{tt}_{tc4}_{doc}")
                    nc.vector.tensor_copy(o_ev[:], f_ps[:])
                    nc.sync.dma_start(out=out[trow:trow + P, doc * 512:(doc + 1) * 512],
                                      in_=o_ev[:])
    nc.compile()
    return nc


def _host_tables():
    freqs = (1.0 / (np.float32(10000.0) **
                    (np.arange(0, HD, 2, dtype=np.float32) / np.float32(HD)))).astype(np.float32)
    t = np.arange(T, dtype=np.float32)
    ang = t[:, None] * freqs[None, :]
    cos = np.tile(np.cos(ang), (1, 2)).astype(np.float32)   # (T, HD)
    sin = np.tile(np.sin(ang), (1, 2)).astype(np.float32)
    cosT = np.ascontiguousarray(cos.T)                       # (HD, T)
    sinT = np.ascontiguousarray(sin.T)
    ssinT = sinT.copy()
    ssinT[:HD // 2] *= -1.0                                  # sign-folded sin
    # pure half-swap permutation; the rotate-half sign lives in ssinT
    rmat = np.zeros((P, P), dtype=np.float32)
    for j in range(HD // 2):
        rmat[j + HD // 2, j] = 1.0
    for j in range(HD // 2, HD):
        rmat[j - HD // 2, j] = 1.0
    tri = (np.arange(P)[:, None] <= np.arange(P)[None, :]).astype(np.float32)
    ident = np.eye(P, dtype=np.float32)
    return cosT, ssinT, rmat, tri, ident


def kernel(x, wq, wk, wv, wo):
    if "nc" not in _cached:
        _cached["nc"] = _build()
    nc = _cached["nc"]
    cosT, ssinT, rmat, tri, ident = _host_tables()
    x = np.asarray(x, dtype=np.float32)
    wq = np.asarray(wq, dtype=np.float32)
    wk = np.asarray(wk, dtype=np.float32)
    wv = np.asarray(wv, dtype=np.float32)
    wo = np.asarray(wo, dtype=np.float32)

    in_maps = []
    for c in range(8):
        b, h = divmod(c, 4)
        in_maps.append({
            "xT": np.ascontiguousarray(x[b].T),
            "wq": np.ascontiguousarray(wq[:, h * 512:(h + 1) * 512]),
            "wk": np.ascontiguousarray(wk[:, h * HD:(h + 1) * HD]),
            "wv": np.ascontiguousarray(wv[:, h * HD:(h + 1) * HD]),
            "wo": np.ascontiguousarray(wo[h * 512:(h + 1) * 512, :]),
            "cosT": cosT, "ssinT": ssinT, "rmat": rmat, "tri": tri, "ident": ident,
        })
    res = run_bass_kernel_spmd(nc, in_maps, core_ids=list(range(8)))
    outs = [res.results[c]["out"] for c in range(8)]
    full = np.stack([outs[0] + outs[1] + outs[2] + outs[3],
                     outs[4] + outs[5] + outs[6] + outs[7]], axis=0)
    return full.astype(np.float32)
